# revision 2
# baseline (speedup 1.0000x reference)
"""Trainium2 Bass kernel v2 for nn_ADI_16389595202112 (moe_routing, 8 cores).

Data-parallel over batch (2048 samples/core, no collectives).
Host: fold BN into weights, fold domain emb into ADI bias (negated for a
single Exp pass), slice domain-d task weights, pack per-side blobs in bf16.

v2 device design (feature-major, per side, weight-stationary sweeps):
- 16 indirect-DMA gathers/side + ONE xbar DMA transpose per half
  (no PE transposes, no DVE copies)
- gate/sigmoid normalizers via one [11,3] selector matmul + 4 small
  reciprocals; all per-sample row broadcasts done by DMA-replicate
  (0-stride free dim), zero engine cost
- evictions round-robined across ACT / DVE / Pool(gpsimd) so PE never
  stalls and the p-state ramp reaches 2.4 GHz
- side0 phase A in 2 passes (overlap gather halves), side1 single-pass
  4-chunk blocks (one LDW per block)
- side0 m3/m4 emitted inside side1 phase A to soak PE while side0's
  share/sshare DVE chain completes
- m3/m4/final use 1-bank [128,512] psum chunks (tag C), phase A/sfc2 use
  [128,1024] tiles (tag A, 3 bufs): 8 banks exactly
"""

import sys

import numpy as np

if "/opt/trn_rl_repo" not in sys.path:
    sys.path.insert(0, "/opt/trn_rl_repo")

import ml_dtypes

import concourse.bass as bass
import concourse.mybir as mybir
import concourse.tile as tile
from concourse import bacc
from concourse.bass import IndirectOffsetOnAxis
from concourse.bass_utils import run_bass_kernel_spmd

DEBUG = False
FILL = False              # PE ramp-keeper dummy matmuls (off: HW throttles)

STT_EXPERTS = ()          # evicted as max(s2,-b)*g, need bias correction

EPS = 1e-5
B, L = 16384, 128
E, T = 8, 5
NUM_ROWS = 100000
NCORES = 8
BC = B // NCORES          # 2048 samples per core
CN = 512                  # matmul N chunk
NCH = BC // CN            # 4 chunks
HB = BC // 2              # 1024 = one psum tile width (2 chunks)
NT = BC // 128            # 16 gather tiles per side

# weight blob column layout (bf16, [128, WCOLS])
W1_OFF = 0                # 2 blocks x 128 (m1)
SFC1_OFF = 256            # 16 blocks x 128; block m=2e+j
GA_OFF = 2304             # [128,10]: cols 0-7 gate_w, cols 8-9 = -adi_w
SFC2_OFF = 2314           # 16 blocks x 128; block (e,kc)=2e+kc
W2_OFF = 4362             # 2 k-blocks x 128 (m2)
W3_OFF = 4618             # 3 k-chunks x 192 (cols 0:128 j0, 128:192 j1)
W4_OFF = 5194             # k0 [128,32] | k1 rows<64 [64,32]
BCOR_OFF = 5258           # [11,128] rows0-7=bsfc2 (bias correction)
GSEL_OFF = 5386           # [11,3]: c0 rows0-7=1 (S); c1 r8,r10=1; c2 r9,r10=1
BLK_OFF = 5389            # [128,4] block-diagonal ones (final reduce)
WCOLS = 5393

# bias blob column layout (f32, [128, BCOLS])
B1_C = 0                  # 2 cols (m1)
BS1_C = 2                 # 16 cols (sfc1)
GAB_C = 18                # rows 0-7 gate_b, rows 8-9 -adi_b_eff
NB2_C = 19                # 8 cols: -bsfc2[e]
NB2M_C = 27               # -b2 (m2)
B2M_C = 28                # +b2 (m2)
B3_C = 29                 # 2 cols (m3 blocks j0/j1)
B4_C = 31                 # b4 replicated per 32-partition group
PB2_C = 32                # 8 cols: +bsfc2[e] (ACT relu path)
BCOLS = 40

F32 = mybir.dt.float32
BF16 = mybir.dt.bfloat16
I32 = mybir.dt.int32

BF = ml_dtypes.bfloat16


def _fold_side(p, prefix, d, dom_emb):
    g = lambda n: np.asarray(p[prefix + n], dtype=np.float64)

    def bn_fold(W, b, bn):
        gamma, beta, mean, var = bn[0], bn[1], bn[2], bn[3]
        s = gamma / np.sqrt(var + EPS)
        return W * s[None, :], (b - mean) * s + beta

    W1, b1 = bn_fold(g("m1_w")[d], g("m1_b")[d], g("bn1")[d])
    W2, b2 = bn_fold(g("m2_w")[d], g("m2_b")[d], g("bn2")[d])
    Wsfc1, bsfc1, Wsfc2, bsfc2 = [], [], [], []
    for e in range(E):
        w, bb = bn_fold(g("sfc1_w")[e], g("sfc1_b")[e], g("dsbn1")[d * E + e])
        Wsfc1.append(w)
        bsfc1.append(bb)
        w, bb = bn_fold(g("sfc2_w")[e], g("sfc2_b")[e], g("dsbn2")[d * E + e])
        Wsfc2.append(w)
        bsfc2.append(bb)
    W3, b3 = bn_fold(g("m3_w")[d], g("m3_b")[d], g("bn3")[d])
    W4, b4 = g("m4_w")[d], g("m4_b")[d]
    adi_b = np.asarray(dom_emb, np.float64) @ g("adi_w") + g("adi_b")

    wblob = np.zeros((128, WCOLS), np.float64)
    wblob[:, W1_OFF:W1_OFF + 256] = W1
    wblob[:, SFC1_OFF:SFC1_OFF + 2048] = np.concatenate(Wsfc1, axis=1)
    wblob[:, GA_OFF:GA_OFF + 8] = g("gate_w")[d]
    wblob[:, GA_OFF + 8:GA_OFF + 10] = -g("adi_w")
    for e in range(E):
        for kc in range(2):
            c0 = SFC2_OFF + (2 * e + kc) * 128
            wblob[:, c0:c0 + 128] = Wsfc2[e][kc * 128:(kc + 1) * 128]
    for kc in range(2):
        wblob[:, W2_OFF + kc * 128:W2_OFF + (kc + 1) * 128] = W2[kc * 128:(kc + 1) * 128]
    for k in range(3):
        wblob[:, W3_OFF + k * 192:W3_OFF + (k + 1) * 192] = W3[k * 128:(k + 1) * 128]
    wblob[:, W4_OFF:W4_OFF + 32] = W4[0:128]
    wblob[0:64, W4_OFF + 32:W4_OFF + 64] = W4[128:192]
    # bias correction only for experts evicted via max(s2,-b) (STT path);
    # ACT-relu experts already include +b
    for e in STT_EXPERTS:
        wblob[e, BCOR_OFF:BCOR_OFF + 128] = bsfc2[e]
    wblob[0:8, GSEL_OFF] = 1.0          # S = sum exp(gate)
    wblob[8, GSEL_OFF + 1] = 1.0        # A0 = 1 + e^-a0
    wblob[10, GSEL_OFF + 1] = 1.0
    wblob[9, GSEL_OFF + 2] = 1.0        # A1 = 1 + e^-a1
    wblob[10, GSEL_OFF + 2] = 1.0
    for c in range(4):
        wblob[32 * c:32 * c + 32, BLK_OFF + c] = 1.0

    bblob = np.zeros((128, BCOLS), np.float64)
    bblob[:, B1_C] = b1[0:128]
    bblob[:, B1_C + 1] = b1[128:256]
    bsfc1 = np.concatenate(bsfc1)
    for m in range(16):
        bblob[:, BS1_C + m] = bsfc1[m * 128:(m + 1) * 128]
    bblob[0:8, GAB_C] = g("gate_b")[d]
    bblob[8:10, GAB_C] = -adi_b
    for e in range(E):
        bblob[:, NB2_C + e] = -bsfc2[e]
        bblob[:, PB2_C + e] = bsfc2[e]
    bblob[:, NB2M_C] = -b2
    bblob[:, B2M_C] = b2
    bblob[:, B3_C] = b3[0:128]
    bblob[0:64, B3_C + 1] = b3[128:192]
    for c in range(4):
        bblob[32 * c:32 * c + 32, B4_C] = b4
    return wblob.astype(BF), bblob.astype(np.float32)


def _build_nc():
    nc = bacc.Bacc(None, target_bir_lowering=False)

    uidx_d = nc.declare_dram_parameter("uidx", [128, NT], I32, isOutput=False)
    iidx_d = nc.declare_dram_parameter("iidx", [128, NT], I32, isOutput=False)
    embu_d = nc.declare_dram_parameter("emb_user", [NUM_ROWS, L], BF16, isOutput=False)
    embi_d = nc.declare_dram_parameter("emb_item", [NUM_ROWS, L], BF16, isOutput=False)
    wu_d = nc.declare_dram_parameter("wu", [128, WCOLS], BF16, isOutput=False)
    wi_d = nc.declare_dram_parameter("wi", [128, WCOLS], BF16, isOutput=False)
    bu_d = nc.declare_dram_parameter("bu", [128, BCOLS], F32, isOutput=False)
    bi_d = nc.declare_dram_parameter("bi", [128, BCOLS], F32, isOutput=False)
    out_d = nc.declare_dram_parameter("out", [4, CN], F32, isOutput=True)
    dbg_d = {}
    if DEBUG:
        for nm, shp, dt in (
            ("d_embT0", [128, NT * 128], BF16), ("d_expg0", [11, BC], BF16),
            ("d_rows0", [3, BC], F32), ("d_corr0", [128, BC], BF16),
            ("d_y1a0", [128, BC], BF16), ("d_s1r0", [128, BC], BF16),
            ("d_m2t0", [128, BC], BF16), ("d_acc0", [128, BC], BF16),
            ("d_share0", [128, BC], BF16), ("d_spec0", [128, BC], BF16),
            ("d_bcg00", [128, BC], BF16), ("d_h3a0", [128, BC], BF16),
            ("d_sout0", [128, CN], BF16), ("d_sout1", [128, CN], BF16),
        ):
            dbg_d[nm] = nc.declare_dram_parameter(nm, shp, dt, isOutput=True)

    A = mybir.ActivationFunctionType
    OP = mybir.AluOpType

    with nc.allow_low_precision(reason="bf16 activations, f32 psum accum"), \
         tile.TileContext(nc) as tc:
        with (
            tc.tile_pool(name="const", bufs=1) as cp,
            tc.tile_pool(name="gath", bufs=2) as gp,
            tc.tile_pool(name="s1", bufs=16) as s1p,
            tc.tile_pool(name="work", bufs=1) as wp,
            tc.tile_pool(name="psA", bufs=3, space="PSUM") as pA,
            tc.tile_pool(name="psC", bufs=1 if FILL else 2, space="PSUM") as pC,
            tc.tile_pool(name="psF", bufs=1, space="PSUM") as pF,
        ):
            # PE ramp-keeper: dependency-free dummy matmuls into a spare
            # psum bank keep the tensor engine's p-state at full clock
            # through short eviction stalls (PE drops to 1.2 GHz after
            # any idle; 2.4 GHz needs 3us of continuous busy).
            fill_state = {"ps": None, "w": None}

            def fill(n):
                if fill_state["ps"] is None or not FILL:
                    return
                for _ in range(n):
                    nc.tensor.matmul(fill_state["ps"][:, 0:256],
                                     fill_state["w"][:, 0:128],
                                     fill_state["w"][:, 0:256],
                                     start=True, stop=True)
            sides = []
            for name, wd, bd, xd, ed in (
                ("u", wu_d, bu_d, uidx_d, embu_d),
                ("i", wi_d, bi_d, iidx_d, embi_d),
            ):
                idx = cp.tile([128, NT], I32, name=f"idx_{name}")
                nc.sync.dma_start(out=idx[:], in_=xd[:, :])
                w = cp.tile([128, WCOLS], BF16, name=f"w_{name}")
                # split weight DMA by first-use section
                for c0, c1 in ((W1_OFF, SFC2_OFF), (BCOR_OFF, WCOLS),
                               (SFC2_OFF, W3_OFF), (W3_OFF, BCOR_OFF)):
                    nc.sync.dma_start(out=w[:, c0:c1], in_=wd[:, c0:c1])
                b = cp.tile([128, BCOLS], F32, name=f"b_{name}")
                nc.sync.dma_start(out=b[:], in_=bd[:, :])
                sout = cp.tile([128, CN], BF16, name=f"sout_{name}")
                sides.append((w, b, idx, ed, sout))

            if FILL:
                fill_state["ps"] = pF.tile([128, 256], F32, name="ps_fill")
                fill_state["w"] = sides[0][0]
                fill(40)  # warm the PE p-state through the gather/xbar head

            embTs = {}
            gts = {}

            def emit_gather(s, half):
                # 8 indirect-DMA row gathers (gpsimd SWDGE), [128,8,128]
                w, b, idx, ed, sout = sides[s]
                gt = gp.tile([128, 8, 128], BF16, tag="g", bufs=2,
                             name=f"g{s}_{half}")
                gts[(s, half)] = gt
                for t in range(8):
                    nc.gpsimd.indirect_dma_start(
                        out=gt[:, t, :], out_offset=None,
                        in_=ed[:, :],
                        in_offset=IndirectOffsetOnAxis(
                            ap=idx[:, 8 * half + t:8 * half + t + 1], axis=0))

            def emit_xbar(s, half):
                # one xbar DMA transpose: [128,1024] -> [128,8,128]
                if s not in embTs:
                    embTs[s] = wp.tile([128, NT, 128], BF16, tag="embT",
                                       bufs=2, name=f"embT{s}")
                gt = gts.pop((s, half))
                nc.sync.dma_start_transpose(
                    out=embTs[s][:, 8 * half:8 * half + 8, :],
                    in_=gt[:, :, :])

            # ---------------- eviction engine round-robin ----------------
            # ACT ~1.07us, DVE ~1.25us, Pool ~2.3us per [128,1024] evict
            def make_rr(pattern):
                state = {"i": 0}

                def next_eng():
                    e = pattern[state["i"] % len(pattern)]
                    state["i"] += 1
                    return e
                return next_eng

            def emit_body(s, first_pass_blocks=None, hooks=None):
                """Emit one side. first_pass_blocks: if set, phase A is
                2-pass (chunks 01 then 23); else single-pass 4-chunk.
                hooks: dict slot -> callable emitted at that point."""
                w, b, idx, ed, sout = sides[s]
                embT = embTs[s]
                hooks = hooks or {}

                def hook(k):
                    for fn in hooks.get(k, ()):
                        fn()

                def dbg(nm, ap):
                    if DEBUG and s == 0:
                        nc.sync.dma_start(out=dbg_d[nm][:, :], in_=ap)

                expg = wp.tile([11, BC], BF16, tag="expg", bufs=1,
                               name=f"expg{s}")
                nc.vector.memset(expg[0:11, :], 1.0)
                y1a = wp.tile([128, BC], BF16, tag="y1a", bufs=1, name=f"y1a{s}")
                y1b = wp.tile([128, BC], BF16, tag="y1b", bufs=1, name=f"y1b{s}")
                s1r = [s1p.tile([128, BC], BF16, tag="s1r", name=f"s1r{s}_{m}")
                       for m in range(16)]

                # negated biases for the DVE/Pool relu trick max(x,-b)+b
                nb = wp.tile([128, BCOLS], F32, tag="nb", bufs=1, name=f"nb{s}")
                nc.vector.tensor_scalar_mul(nb[:, 0:BCOLS], b[:, 0:BCOLS], -1.0)

                def ev_relu(eng, dst, cols, bc, ps, wd):
                    # dst[:, cols] = relu(ps + b[:, bc])
                    if eng == "A":
                        nc.scalar.activation(dst[:, cols], ps[:, 0:wd],
                                             A.Relu, bias=b[:, bc:bc + 1])
                    else:
                        en = nc.vector if eng == "V" else nc.gpsimd
                        en.tensor_scalar(dst[:, cols], ps[:, 0:wd],
                                         nb[:, bc:bc + 1], b[:, bc:bc + 1],
                                         op0=OP.max, op1=OP.add)

                # ---- phase A ----
                # block list: (wcol, mwidth, evict_fn)
                def ev_ga(ps, wd, c0, c1):
                    nc.scalar.activation(
                        expg[0:10, c0:c1], ps[0:10, 0:wd], A.Exp,
                        bias=b[0:10, GAB_C:GAB_C + 1])

                def ablock(wc, mw, evict, chunks, rr):
                    # chunks: tuple of chunk ids; one psum tile per 2 chunks
                    for g0 in range(0, len(chunks), 2):
                        pair = chunks[g0:g0 + 2]
                        wd = len(pair) * CN
                        ps = pA.tile([128, HB], F32, tag="A", name="ps_a")
                        for h, cc in enumerate(pair):
                            nc.tensor.matmul(ps[0:mw, h * CN:(h + 1) * CN],
                                             w[:, wc:wc + mw],
                                             embT[:, 4 * cc:4 * cc + 4, :],
                                             start=True, stop=True)
                        fill(len(pair))
                        evict(ps, wd, pair[0] * CN, (pair[-1] + 1) * CN)

                # broadcast plumbing declared early so the expg-dependent
                # replicates can be emitted as soon as expg is complete
                dram_bc = nc.dram_tensor(f"bcast{s}", [11, BC], BF16,
                                         kind="Internal")
                bcg = [wp.tile([128, BC], BF16, tag="bcg", bufs=6,
                               name=f"bcg{s}_{e}") for e in range(E)]
                bR = wp.tile([128, 3, BC], BF16, tag="bR", bufs=1,
                             name=f"bR{s}")

                def replicate(dst, r):
                    nc.sync.dma_start(
                        out=dst,
                        in_=dram_bc[r:r + 1, :].unsqueeze(1).broadcast_to(
                            (1, 128, BC)))

                def gate_head():
                    nc.sync.dma_start(out=dram_bc[0:8, :], in_=expg[0:8, :])
                    for e in range(6):
                        replicate(bcg[e][:], e)
                    # bcg[6..7] reuse slots freed only mid-sweep; anything
                    # queued after them would stall, so they are emitted
                    # after the bR replicates (gate-normalizer section)
                    hook("midbcg")

                def phaseA(chunks, rr_pattern, ga=True, m1=True,
                           ms=range(16)):
                    rr = make_rr(rr_pattern)

                    def ev_y(dst, bc):
                        def ev(ps, wd, c0, c1):
                            ev_relu(rr(), dst, slice(c0, c1), bc, ps, wd)
                        return ev

                    if ga:
                        ablock(GA_OFF, 10, ev_ga, chunks, rr)
                        hook("postGA" + str(chunks[0]))
                    if m1:
                        ablock(W1_OFF, 128, ev_y(y1a, B1_C), chunks, rr)
                        ablock(W1_OFF + 128, 128, ev_y(y1b, B1_C + 1),
                               chunks, rr)
                    for m in ms:
                        ablock(SFC1_OFF + m * 128, 128,
                               ev_y(s1r[m], BS1_C + m), chunks, rr)
                        hook(f"sfc1_{chunks[0]}_{m}")

                def ga_late():
                    # second-half GA as soon as embT half 1 is live, so
                    # the expg broadcasts start mid phase A
                    ablock(GA_OFF, 10, ev_ga, (2, 3), None)
                    gate_head()

                if first_pass_blocks is not None:
                    # single-chunk prefix passes: start compute after only
                    # 4 gather tiles instead of 8
                    hooks.setdefault("sfc1_0_10", []).append(ga_late)
                    phaseA((0,), first_pass_blocks, ms=range(0, 6))
                    phaseA((1,), first_pass_blocks, ms=range(0, 6))
                    phaseA((0, 1), first_pass_blocks, ga=False, m1=False,
                           ms=range(6, 16))
                    hook("midA")
                    phaseA((2, 3), first_pass_blocks, ga=False)
                else:
                    hooks.setdefault("postGA0", []).append(gate_head)
                    phaseA((0, 1, 2, 3), ("A", "V"))
                hook("postA")

                # ---- gate normalizers (after expg complete) ----
                # corr = bsfc2 @ expg, only needed if any STT experts
                corr = None
                if STT_EXPERTS:
                    corr = wp.tile([128, BC], BF16, tag="corr", bufs=1,
                                   name=f"corr{s}")
                    for p in range(2):
                        ps = pA.tile([128, HB], F32, tag="A", name="ps_cor")
                        for h in range(2):
                            c0 = (2 * p + h) * CN
                            nc.tensor.matmul(ps[:, h * CN:(h + 1) * CN],
                                             w[0:11, BCOR_OFF:BCOR_OFF + 128],
                                             expg[0:11, c0:c0 + CN],
                                             start=True, stop=True)
                        nc.scalar.activation(corr[:, p * HB:(p + 1) * HB],
                                             ps[:], A.Copy)
                dbg("d_embT0", embT[:, :, :])
                dbg("d_expg0", expg[:, :])
                dbg("d_y1a0", y1a[:, :])
                dbg("d_s1r0", s1r[0][:, :])

                # gsel: [3, BC] rows S, A0, A1 in 4 chunk psums -> recips
                rows32 = wp.tile([3, BC], F32, tag="rows32", bufs=1,
                                 name=f"rows32_{s}")
                rows = wp.tile([3, BC], BF16, tag="rows", bufs=1,
                               name=f"rows{s}")
                for c in range(NCH):
                    ps = pC.tile([128, CN], F32, tag="C", name="ps_gs")
                    nc.tensor.matmul(ps[0:3, 0:CN],
                                     w[0:11, GSEL_OFF:GSEL_OFF + 3],
                                     expg[0:11, c * CN:(c + 1) * CN],
                                     start=True, stop=True)
                    nc.vector.reciprocal_approx_fast(
                        rows32[:, c * CN:(c + 1) * CN], ps[0:3, 0:CN])
                nc.scalar.activation(rows[:], rows32[:], A.Copy)
                dbg("d_rows0", rows32[:, :])
                nc.sync.dma_start(out=dram_bc[8:11, :], in_=rows[:, :])
                for r in range(3):
                    replicate(bR[:, r, :], 8 + r)
                for e in range(6, E):
                    replicate(bcg[e][:], e)
                dbg("d_bcg00", bcg[0][:, :])

                def kblock(wcols, movings, evict, mw=128):
                    # stationary-major: 2 psum tiles alternate; accumulate
                    # over k-chunks
                    pss = [pA.tile([128, HB], F32, tag="A", name="ps_k")
                           for _ in range(2)]
                    for kc, wc in enumerate(wcols):
                        for c in range(NCH):
                            ps = pss[c // 2]
                            nc.tensor.matmul(
                                ps[0:mw, (c % 2) * CN:(c % 2 + 1) * CN],
                                w[:, wc:wc + mw],
                                movings[kc][:, c * CN:(c + 1) * CN],
                                start=(kc == 0), stop=(kc == len(wcols) - 1))
                    fill(2)
                    for p in range(2):
                        evict(pss[p], p)

                # ---- m2 ----
                m2t = wp.tile([128, BC], BF16, tag="m2t", bufs=1,
                              name=f"m2t{s}")

                def ev_m2(ps, p):
                    if p == 0:
                        nc.scalar.activation(
                            m2t[:, 0:HB], ps[:], A.Relu,
                            bias=b[:, B2M_C:B2M_C + 1])
                    else:
                        nc.vector.tensor_scalar(
                            m2t[:, HB:BC], ps[:],
                            b[:, NB2M_C:NB2M_C + 1], b[:, B2M_C:B2M_C + 1],
                            op0=OP.max, op1=OP.add)

                kblock([W2_OFF, W2_OFF + 128], [y1a, y1b], ev_m2)
                dbg("d_m2t0", m2t[:, :])
                hook("postm2")

                # ---- sfc2 expert sweep + gated eager pairwise sum ----
                acc = wp.tile([128, BC], BF16, tag="acc", bufs=1,
                              name=f"acc{s}")
                pe_t = [wp.tile([128, BC], BF16, tag="pe", bufs=2,
                                name=f"pe{s}_{i}") for i in range(2)]
                for e in range(E):
                    pt = acc if e == 0 else pe_t[e % 2]

                    def ev_pe(ps, p, e=e, pt=pt):
                        # Pool cannot read PSUM. Experts 0-5: ACT relu
                        # eviction in place, then one full-width gate mul
                        # on Pool (e 0-2) / DVE (e 3-5). Experts 6-7:
                        # fused DVE scalar_tensor_tensor.
                        if e not in STT_EXPERTS:
                            nc.scalar.activation(
                                pt[:, p * HB:(p + 1) * HB], ps[:], A.Relu,
                                bias=b[:, PB2_C + e:PB2_C + e + 1])
                        else:
                            nc.vector.scalar_tensor_tensor(
                                pt[:, p * HB:(p + 1) * HB], ps[:],
                                b[:, NB2_C + e:NB2_C + e + 1],
                                bcg[e][:, p * HB:(p + 1) * HB],
                                op0=OP.max, op1=OP.mult)

                    kblock([SFC2_OFF + (2 * e) * 128,
                            SFC2_OFF + (2 * e + 1) * 128],
                           [s1r[2 * e], s1r[2 * e + 1]], ev_pe)
                    if e not in STT_EXPERTS:
                        en = nc.gpsimd if e < 3 else nc.vector
                        en.tensor_mul(pt[:], pt[:], bcg[e][:])
                    if e > 0:
                        nc.vector.tensor_add(acc[:], acc[:], pt[:])
                    hook(f"sfc2_{e}")
                if corr is not None:
                    nc.vector.tensor_add(acc[:], acc[:], corr[:])
                dbg("d_acc0", acc[:, :])
                fill(16)  # keep PE warm through the share/sshare chain
                # spec emitted after the sweep: it waits on bR, and at the
                # head of the DVE queue it would block every sweep DVE op
                spec = wp.tile([128, BC], BF16, tag="spec", bufs=1,
                               name=f"spec{s}")
                nc.vector.tensor_mul(spec[:], m2t[:], bR[:, 2, :])
                dbg("d_spec0", spec[:, :])

                # share = acc * (1/S) * (1/A0) ; sshare = spec * share
                tmp = pe_t[0]
                nc.vector.tensor_mul(tmp[:], acc[:], bR[:, 0, :])
                share = wp.tile([128, BC], BF16, tag="share", bufs=1,
                                name=f"share{s}")
                nc.vector.tensor_mul(share[:], tmp[:], bR[:, 1, :])
                sshare = wp.tile([128, BC], BF16, tag="sshare", bufs=1,
                                 name=f"ssh{s}")
                nc.vector.tensor_mul(sshare[:], spec[:], share[:])
                dbg("d_share0", share[:, :])
                hook("postchain")
                return spec, share, sshare, nb

            def emit_tail(s, spec, share, sshare, nb, engs=("A", "A", "A")):
                # ---- m3 (1-bank psum chunks) + m4 + sout ----
                w, b, idx, ed, sout = sides[s]
                movs = ((spec, 0), (share, 2), (sshare, 1))
                h3a = wp.tile([128, BC], BF16, tag="h3a", bufs=1,
                              name=f"h3a{s}")
                h3b = wp.tile([64, BC], BF16, tag="h3b", bufs=1,
                              name=f"h3b{s}")

                def ev(eng, dst, ps, mw, bc):
                    if eng == "A":
                        nc.scalar.activation(dst, ps, A.Relu,
                                             bias=b[0:mw, bc:bc + 1])
                    else:
                        nc.vector.tensor_scalar(dst, ps,
                                                nb[0:mw, bc:bc + 1],
                                                b[0:mw, bc:bc + 1],
                                                op0=OP.max, op1=OP.add)

                for j, mw, dst in ((0, 128, h3a), (1, 64, h3b)):
                    for c in range(NCH):
                        ps = pC.tile([128, CN], F32, tag="C", name="ps_m3")
                        for ki, (mv, k) in enumerate(movs):
                            nc.tensor.matmul(
                                ps[0:mw, 0:CN],
                                w[:, W3_OFF + k * 192 + j * 128:
                                  W3_OFF + k * 192 + j * 128 + mw],
                                mv[:, c * CN:(c + 1) * CN],
                                start=(ki == 0), stop=(ki == 2))
                        ev(engs[j], dst[0:mw, c * CN:(c + 1) * CN],
                           ps[0:mw, 0:CN], mw, B3_C + j)

                ps4 = pC.tile([128, CN], F32, tag="C", name="ps_m4")
                for kc, (kw, wc, src) in enumerate(
                        ((128, W4_OFF, h3a), (64, W4_OFF + 32, h3b))):
                    for c in range(NCH):
                        nc.tensor.matmul(
                            ps4[32 * c:32 * c + 32, 0:CN],
                            w[0:kw, wc:wc + 32],
                            src[0:kw, c * CN:(c + 1) * CN],
                            start=(kc == 0), stop=(kc == 1),
                            tile_position=(0, 32 * c))
                ev(engs[2], sout[:], ps4[:, 0:CN], 128, B4_C)
                if DEBUG and s == 0:
                    nc.sync.dma_start(out=dbg_d["d_h3a0"][:, :], in_=h3a[:, :])
                if DEBUG:
                    nc.sync.dma_start(out=dbg_d[f"d_sout{s}"][:, :],
                                      in_=sout[:, :])

            # ================= global schedule =================
            # all gathers queued up front on gpsimd
            emit_gather(0, 0)
            emit_gather(0, 1)
            emit_gather(1, 0)
            emit_gather(1, 1)
            emit_xbar(0, 0)
            emit_xbar(0, 1)
            # gt pool bufs=2: side-1's gathers reuse side-0's slots only
            # after the side-0 xbars read them

            # side 0: 2-pass phase A (Pool busy with gathers -> ACT/DVE).
            # side-1 xbars emitted at postA: after gate_head's replicate
            # DMAs (sync queue) but before the bR replicates.
            r0 = emit_body(0, first_pass_blocks=("A", "V"), hooks={
                "midbcg": [lambda: [emit_xbar(1, h) for h in range(2)]]})
            # side 1: phase A first blocks fill PE while side0 chain runs
            tail0 = {"sfc1_0_2": (lambda: emit_tail(
                0, *r0, engs=("V", "A", "A")),)}
            r1 = emit_body(1, hooks=tail0)
            emit_tail(1, *r1, engs=("A", "A", "A"))

            # ---- logits: block-diagonal reduce of sout_u * sout_i ----
            su, si = sides[0][4], sides[1][4]
            prod = wp.tile([128, CN], BF16, tag="prod", bufs=1)
            nc.vector.tensor_mul(prod[:], su[:], si[:])
            psf = pC.tile([128, CN], F32, tag="C", name="ps_f")
            nc.tensor.matmul(psf[0:4, 0:CN],
                             sides[0][0][:, BLK_OFF:BLK_OFF + 4],
                             prod[:], start=True, stop=True)
            outsb = cp.tile([4, CN], F32)
            nc.scalar.activation(outsb[:], psf[0:4, 0:CN], A.Copy)
            nc.sync.dma_start(out=out_d[:, :], in_=outsb[:])

    nc.finalize()
    return nc


def _make_in_maps(inputs):
    d = int(inputs["domain_idc"])
    wu, bu = _fold_side(inputs, "u_", d, np.asarray(inputs["domain_embs"])[d])
    wi, bi = _fold_side(inputs, "i_", d, np.asarray(inputs["domain_embs"])[d + T])
    v = lambda x: x.view(np.uint16)
    embu = v(np.ascontiguousarray(np.asarray(inputs["emb_user"], np.float32).astype(BF)))
    embi = v(np.ascontiguousarray(np.asarray(inputs["emb_item"], np.float32).astype(BF)))
    uidx = np.asarray(inputs["user_indices"], np.int32)
    iidx = np.asarray(inputs["item_indices"], np.int32)

    in_maps = []
    for c in range(NCORES):
        sl = slice(c * BC, (c + 1) * BC)
        in_maps.append({
            "uidx": np.ascontiguousarray(uidx[sl].reshape(NT, 128).T),
            "iidx": np.ascontiguousarray(iidx[sl].reshape(NT, 128).T),
            "emb_user": embu,
            "emb_item": embi,
            "wu": v(wu), "wi": v(wi), "bu": bu, "bi": bi,
        })
    return in_maps


_CACHED_NC = None


def _get_nc():
    global _CACHED_NC
    if _CACHED_NC is None:
        _CACHED_NC = _build_nc()
    return _CACHED_NC


def run(inputs, **kw):
    """Run on 8 cores; returns (full_output, BassKernelResults)."""
    res = run_bass_kernel_spmd(_get_nc(), _make_in_maps(inputs),
                               core_ids=list(range(NCORES)), **kw)
    out = np.concatenate([res.results[c]["out"].reshape(-1)
                          for c in range(NCORES)])
    return out.reshape(B, 1).astype(np.float32), res


def kernel(**inputs):
    out, _ = run(inputs)
    return out


# revision 3
# speedup vs baseline: 1.0298x; 1.0298x over previous
"""Trainium2 Bass kernel v2 for nn_ADI_16389595202112 (moe_routing, 8 cores).

Data-parallel over batch (2048 samples/core, no collectives).
Host: fold BN into weights, fold domain emb into ADI bias (negated for a
single Exp pass), slice domain-d task weights, pack per-side blobs in bf16.

v2 device design (feature-major, per side, weight-stationary sweeps):
- 16 indirect-DMA gathers/side + ONE xbar DMA transpose per half
  (no PE transposes, no DVE copies)
- gate/sigmoid normalizers via one [11,3] selector matmul + 4 small
  reciprocals; all per-sample row broadcasts done by DMA-replicate
  (0-stride free dim), zero engine cost
- evictions round-robined across ACT / DVE / Pool(gpsimd) so PE never
  stalls and the p-state ramp reaches 2.4 GHz
- side0 phase A in 2 passes (overlap gather halves), side1 single-pass
  4-chunk blocks (one LDW per block)
- side0 m3/m4 emitted inside side1 phase A to soak PE while side0's
  share/sshare DVE chain completes
- m3/m4/final use 1-bank [128,512] psum chunks (tag C), phase A/sfc2 use
  [128,1024] tiles (tag A, 3 bufs): 8 banks exactly
"""

import sys

import numpy as np

if "/opt/trn_rl_repo" not in sys.path:
    sys.path.insert(0, "/opt/trn_rl_repo")

import ml_dtypes

import concourse.bass as bass
import concourse.mybir as mybir
import concourse.tile as tile
from concourse import bacc
from concourse.bass import IndirectOffsetOnAxis
from concourse.bass_utils import run_bass_kernel_spmd

DEBUG = False
FILL = False              # PE ramp-keeper dummy matmuls (off: HW throttles)

STT_EXPERTS = ()          # evicted as max(s2,-b)*g, need bias correction

EPS = 1e-5
B, L = 16384, 128
E, T = 8, 5
NUM_ROWS = 100000
NCORES = 8
BC = B // NCORES          # 2048 samples per core
CN = 512                  # matmul N chunk
NCH = BC // CN            # 4 chunks
HB = BC // 2              # 1024 = one psum tile width (2 chunks)
NT = BC // 128            # 16 gather tiles per side

# weight blob column layout (bf16, [128, WCOLS])
W1_OFF = 0                # 2 blocks x 128 (m1)
SFC1_OFF = 256            # 16 blocks x 128; block m=2e+j
GA_OFF = 2304             # [128,10]: cols 0-7 gate_w, cols 8-9 = -adi_w
SFC2_OFF = 2314           # 16 blocks x 128; block (e,kc)=2e+kc
W2_OFF = 4362             # 2 k-blocks x 128 (m2)
W3_OFF = 4618             # 3 k-chunks x 192 (cols 0:128 j0, 128:192 j1)
W4_OFF = 5194             # k0 [128,32] | k1 rows<64 [64,32]
BCOR_OFF = 5258           # [11,128] rows0-7=bsfc2 (bias correction)
GSEL_OFF = 5386           # [11,3]: c0 rows0-7=1 (S); c1 r8,r10=1; c2 r9,r10=1
BLK_OFF = 5389            # [128,4] block-diagonal ones (final reduce)
WCOLS = 5393

# bias blob column layout (f32, [128, BCOLS])
B1_C = 0                  # 2 cols (m1)
BS1_C = 2                 # 16 cols (sfc1)
GAB_C = 18                # rows 0-7 gate_b, rows 8-9 -adi_b_eff
NB2_C = 19                # 8 cols: -bsfc2[e]
NB2M_C = 27               # -b2 (m2)
B2M_C = 28                # +b2 (m2)
B3_C = 29                 # 2 cols (m3 blocks j0/j1)
B4_C = 31                 # b4 replicated per 32-partition group
PB2_C = 32                # 8 cols: +bsfc2[e] (ACT relu path)
BCOLS = 40

F32 = mybir.dt.float32
BF16 = mybir.dt.bfloat16
I32 = mybir.dt.int32

BF = ml_dtypes.bfloat16


def _fold_side(p, prefix, d, dom_emb):
    g = lambda n: np.asarray(p[prefix + n], dtype=np.float64)

    def bn_fold(W, b, bn):
        gamma, beta, mean, var = bn[0], bn[1], bn[2], bn[3]
        s = gamma / np.sqrt(var + EPS)
        return W * s[None, :], (b - mean) * s + beta

    W1, b1 = bn_fold(g("m1_w")[d], g("m1_b")[d], g("bn1")[d])
    W2, b2 = bn_fold(g("m2_w")[d], g("m2_b")[d], g("bn2")[d])
    Wsfc1, bsfc1, Wsfc2, bsfc2 = [], [], [], []
    for e in range(E):
        w, bb = bn_fold(g("sfc1_w")[e], g("sfc1_b")[e], g("dsbn1")[d * E + e])
        Wsfc1.append(w)
        bsfc1.append(bb)
        w, bb = bn_fold(g("sfc2_w")[e], g("sfc2_b")[e], g("dsbn2")[d * E + e])
        Wsfc2.append(w)
        bsfc2.append(bb)
    W3, b3 = bn_fold(g("m3_w")[d], g("m3_b")[d], g("bn3")[d])
    W4, b4 = g("m4_w")[d], g("m4_b")[d]
    adi_b = np.asarray(dom_emb, np.float64) @ g("adi_w") + g("adi_b")

    wblob = np.zeros((128, WCOLS), np.float64)
    wblob[:, W1_OFF:W1_OFF + 256] = W1
    wblob[:, SFC1_OFF:SFC1_OFF + 2048] = np.concatenate(Wsfc1, axis=1)
    wblob[:, GA_OFF:GA_OFF + 8] = g("gate_w")[d]
    wblob[:, GA_OFF + 8:GA_OFF + 10] = -g("adi_w")
    for e in range(E):
        for kc in range(2):
            c0 = SFC2_OFF + (2 * e + kc) * 128
            wblob[:, c0:c0 + 128] = Wsfc2[e][kc * 128:(kc + 1) * 128]
    for kc in range(2):
        wblob[:, W2_OFF + kc * 128:W2_OFF + (kc + 1) * 128] = W2[kc * 128:(kc + 1) * 128]
    for k in range(3):
        wblob[:, W3_OFF + k * 192:W3_OFF + (k + 1) * 192] = W3[k * 128:(k + 1) * 128]
    wblob[:, W4_OFF:W4_OFF + 32] = W4[0:128]
    wblob[0:64, W4_OFF + 32:W4_OFF + 64] = W4[128:192]
    # bias correction only for experts evicted via max(s2,-b) (STT path);
    # ACT-relu experts already include +b
    for e in STT_EXPERTS:
        wblob[e, BCOR_OFF:BCOR_OFF + 128] = bsfc2[e]
    wblob[0:8, GSEL_OFF] = 1.0          # S = sum exp(gate)
    wblob[8, GSEL_OFF + 1] = 1.0        # A0 = 1 + e^-a0
    wblob[10, GSEL_OFF + 1] = 1.0
    wblob[9, GSEL_OFF + 2] = 1.0        # A1 = 1 + e^-a1
    wblob[10, GSEL_OFF + 2] = 1.0
    for c in range(4):
        wblob[32 * c:32 * c + 32, BLK_OFF + c] = 1.0

    bblob = np.zeros((128, BCOLS), np.float64)
    bblob[:, B1_C] = b1[0:128]
    bblob[:, B1_C + 1] = b1[128:256]
    bsfc1 = np.concatenate(bsfc1)
    for m in range(16):
        bblob[:, BS1_C + m] = bsfc1[m * 128:(m + 1) * 128]
    bblob[0:8, GAB_C] = g("gate_b")[d]
    bblob[8:10, GAB_C] = -adi_b
    for e in range(E):
        bblob[:, NB2_C + e] = -bsfc2[e]
        bblob[:, PB2_C + e] = bsfc2[e]
    bblob[:, NB2M_C] = -b2
    bblob[:, B2M_C] = b2
    bblob[:, B3_C] = b3[0:128]
    bblob[0:64, B3_C + 1] = b3[128:192]
    for c in range(4):
        bblob[32 * c:32 * c + 32, B4_C] = b4
    return wblob.astype(BF), bblob.astype(np.float32)


def _build_nc():
    nc = bacc.Bacc(None, target_bir_lowering=False)

    uidx_d = nc.declare_dram_parameter("uidx", [128, NT], I32, isOutput=False)
    iidx_d = nc.declare_dram_parameter("iidx", [128, NT], I32, isOutput=False)
    embu_d = nc.declare_dram_parameter("emb_user", [NUM_ROWS, L], BF16, isOutput=False)
    embi_d = nc.declare_dram_parameter("emb_item", [NUM_ROWS, L], BF16, isOutput=False)
    wu_d = nc.declare_dram_parameter("wu", [128, WCOLS], BF16, isOutput=False)
    wi_d = nc.declare_dram_parameter("wi", [128, WCOLS], BF16, isOutput=False)
    bu_d = nc.declare_dram_parameter("bu", [128, BCOLS], F32, isOutput=False)
    bi_d = nc.declare_dram_parameter("bi", [128, BCOLS], F32, isOutput=False)
    out_d = nc.declare_dram_parameter("out", [4, CN], F32, isOutput=True)
    dbg_d = {}
    if DEBUG:
        for nm, shp, dt in (
            ("d_embT0", [128, NT * 128], BF16), ("d_expg0", [11, BC], BF16),
            ("d_rows0", [3, BC], F32), ("d_corr0", [128, BC], BF16),
            ("d_y1a0", [128, BC], BF16), ("d_s1r0", [128, BC], BF16),
            ("d_m2t0", [128, BC], BF16), ("d_acc0", [128, BC], BF16),
            ("d_share0", [128, BC], BF16), ("d_spec0", [128, BC], BF16),
            ("d_bcg00", [128, BC], BF16), ("d_h3a0", [128, BC], BF16),
            ("d_sout0", [128, CN], BF16), ("d_sout1", [128, CN], BF16),
        ):
            dbg_d[nm] = nc.declare_dram_parameter(nm, shp, dt, isOutput=True)

    A = mybir.ActivationFunctionType
    OP = mybir.AluOpType

    with nc.allow_low_precision(reason="bf16 activations, f32 psum accum"), \
         tile.TileContext(nc) as tc:
        with (
            tc.tile_pool(name="const", bufs=1) as cp,
            tc.tile_pool(name="gath", bufs=2) as gp,
            tc.tile_pool(name="s1", bufs=16) as s1p,
            tc.tile_pool(name="work", bufs=1) as wp,
            tc.tile_pool(name="psA", bufs=3, space="PSUM") as pA,
            tc.tile_pool(name="psC", bufs=1 if FILL else 2, space="PSUM") as pC,
            tc.tile_pool(name="psF", bufs=1, space="PSUM") as pF,
        ):
            # PE ramp-keeper: dependency-free dummy matmuls into a spare
            # psum bank keep the tensor engine's p-state at full clock
            # through short eviction stalls (PE drops to 1.2 GHz after
            # any idle; 2.4 GHz needs 3us of continuous busy).
            fill_state = {"ps": None, "w": None}

            def fill(n):
                if fill_state["ps"] is None or not FILL:
                    return
                for _ in range(n):
                    nc.tensor.matmul(fill_state["ps"][:, 0:256],
                                     fill_state["w"][:, 0:128],
                                     fill_state["w"][:, 0:256],
                                     start=True, stop=True)
            sides = []
            for name, wd, bd, xd, ed in (
                ("u", wu_d, bu_d, uidx_d, embu_d),
                ("i", wi_d, bi_d, iidx_d, embi_d),
            ):
                idx = cp.tile([128, NT], I32, name=f"idx_{name}")
                nc.sync.dma_start(out=idx[:], in_=xd[:, :])
                w = cp.tile([128, WCOLS], BF16, name=f"w_{name}")
                # split weight DMA by first-use section
                for c0, c1 in ((W1_OFF, SFC2_OFF), (BCOR_OFF, WCOLS),
                               (SFC2_OFF, W3_OFF), (W3_OFF, BCOR_OFF)):
                    nc.sync.dma_start(out=w[:, c0:c1], in_=wd[:, c0:c1])
                b = cp.tile([128, BCOLS], F32, name=f"b_{name}")
                nc.sync.dma_start(out=b[:], in_=bd[:, :])
                sout = cp.tile([128, CN], BF16, name=f"sout_{name}")
                sides.append((w, b, idx, ed, sout))

            if FILL:
                fill_state["ps"] = pF.tile([128, 256], F32, name="ps_fill")
                fill_state["w"] = sides[0][0]
                fill(40)  # warm the PE p-state through the gather/xbar head

            embTs = {}
            gts = {}

            def emit_gather(s, half):
                # 8 indirect-DMA row gathers (gpsimd SWDGE), [128,8,128]
                w, b, idx, ed, sout = sides[s]
                gt = gp.tile([128, 8, 128], BF16, tag="g", bufs=2,
                             name=f"g{s}_{half}")
                gts[(s, half)] = gt
                for t in range(8):
                    nc.gpsimd.indirect_dma_start(
                        out=gt[:, t, :], out_offset=None,
                        in_=ed[:, :],
                        in_offset=IndirectOffsetOnAxis(
                            ap=idx[:, 8 * half + t:8 * half + t + 1], axis=0))

            def emit_xbar(s, half, split=False):
                # one xbar DMA transpose: [128,1024] -> [128,8,128];
                # split=True emits two quarter transposes so compute can
                # start after only 4 gather tiles
                if s not in embTs:
                    embTs[s] = wp.tile([128, NT, 128], BF16, tag="embT",
                                       bufs=2, name=f"embT{s}")
                gt = gts.pop((s, half))
                if split:
                    for q in range(2):
                        nc.sync.dma_start_transpose(
                            out=embTs[s][:, 8 * half + 4 * q:
                                         8 * half + 4 * q + 4, :],
                            in_=gt[:, 4 * q:4 * q + 4, :])
                else:
                    nc.sync.dma_start_transpose(
                        out=embTs[s][:, 8 * half:8 * half + 8, :],
                        in_=gt[:, :, :])

            # ---------------- eviction engine round-robin ----------------
            # ACT ~1.07us, DVE ~1.25us, Pool ~2.3us per [128,1024] evict
            def make_rr(pattern):
                state = {"i": 0}

                def next_eng():
                    e = pattern[state["i"] % len(pattern)]
                    state["i"] += 1
                    return e
                return next_eng

            def emit_body(s, first_pass_blocks=None, hooks=None):
                """Emit one side. first_pass_blocks: if set, phase A is
                2-pass (chunks 01 then 23); else single-pass 4-chunk.
                hooks: dict slot -> callable emitted at that point."""
                w, b, idx, ed, sout = sides[s]
                embT = embTs[s]
                hooks = hooks or {}

                def hook(k):
                    for fn in hooks.get(k, ()):
                        fn()

                def dbg(nm, ap):
                    if DEBUG and s == 0:
                        nc.sync.dma_start(out=dbg_d[nm][:, :], in_=ap)

                expg = wp.tile([11, BC], BF16, tag="expg", bufs=1,
                               name=f"expg{s}")
                nc.vector.memset(expg[0:11, :], 1.0)
                y1a = wp.tile([128, BC], BF16, tag="y1a", bufs=1, name=f"y1a{s}")
                y1b = wp.tile([128, BC], BF16, tag="y1b", bufs=1, name=f"y1b{s}")
                s1r = [s1p.tile([128, BC], BF16, tag="s1r", name=f"s1r{s}_{m}")
                       for m in range(16)]

                # negated biases for the DVE/Pool relu trick max(x,-b)+b
                nb = wp.tile([128, BCOLS], F32, tag="nb", bufs=1, name=f"nb{s}")
                nc.vector.tensor_scalar_mul(nb[:, 0:BCOLS], b[:, 0:BCOLS], -1.0)

                def ev_relu(eng, dst, cols, bc, ps, wd):
                    # dst[:, cols] = relu(ps + b[:, bc])
                    if eng == "A":
                        nc.scalar.activation(dst[:, cols], ps[:, 0:wd],
                                             A.Relu, bias=b[:, bc:bc + 1])
                    else:
                        en = nc.vector if eng == "V" else nc.gpsimd
                        en.tensor_scalar(dst[:, cols], ps[:, 0:wd],
                                         nb[:, bc:bc + 1], b[:, bc:bc + 1],
                                         op0=OP.max, op1=OP.add)

                # ---- phase A ----
                # block list: (wcol, mwidth, evict_fn)
                def ev_ga(ps, wd, c0, c1):
                    nc.scalar.activation(
                        expg[0:10, c0:c1], ps[0:10, 0:wd], A.Exp,
                        bias=b[0:10, GAB_C:GAB_C + 1])

                def ablock(wc, mw, evict, chunks, rr):
                    # chunks: tuple of chunk ids; one psum tile per 2 chunks
                    for g0 in range(0, len(chunks), 2):
                        pair = chunks[g0:g0 + 2]
                        wd = len(pair) * CN
                        ps = pA.tile([128, HB], F32, tag="A", name="ps_a")
                        for h, cc in enumerate(pair):
                            nc.tensor.matmul(ps[0:mw, h * CN:(h + 1) * CN],
                                             w[:, wc:wc + mw],
                                             embT[:, 4 * cc:4 * cc + 4, :],
                                             start=True, stop=True)
                        fill(len(pair))
                        evict(ps, wd, pair[0] * CN, (pair[-1] + 1) * CN)

                # broadcast plumbing declared early so the expg-dependent
                # replicates can be emitted as soon as expg is complete
                dram_bc = nc.dram_tensor(f"bcast{s}", [11, BC], BF16,
                                         kind="Internal")
                bcg = [wp.tile([128, BC], BF16, tag="bcg", bufs=6,
                               name=f"bcg{s}_{e}") for e in range(E)]
                bR = wp.tile([128, 3, BC], BF16, tag="bR", bufs=1,
                             name=f"bR{s}")

                def replicate(dst, r):
                    nc.sync.dma_start(
                        out=dst,
                        in_=dram_bc[r:r + 1, :].unsqueeze(1).broadcast_to(
                            (1, 128, BC)))

                def gate_head():
                    nc.sync.dma_start(out=dram_bc[0:8, :], in_=expg[0:8, :])
                    for e in range(6):
                        replicate(bcg[e][:], e)
                    # bcg[6..7] reuse slots freed only mid-sweep; anything
                    # queued after them would stall, so they are emitted
                    # after the bR replicates (gate-normalizer section)
                    hook("midbcg")

                def phaseA(chunks, rr_pattern, ga=True, m1=True,
                           ms=range(16)):
                    rr = make_rr(rr_pattern)

                    def ev_y(dst, bc):
                        def ev(ps, wd, c0, c1):
                            ev_relu(rr(), dst, slice(c0, c1), bc, ps, wd)
                        return ev

                    if ga:
                        ablock(GA_OFF, 10, ev_ga, chunks, rr)
                        hook("postGA" + str(chunks[0]))
                    if m1:
                        ablock(W1_OFF, 128, ev_y(y1a, B1_C), chunks, rr)
                        ablock(W1_OFF + 128, 128, ev_y(y1b, B1_C + 1),
                               chunks, rr)
                    for m in ms:
                        ablock(SFC1_OFF + m * 128, 128,
                               ev_y(s1r[m], BS1_C + m), chunks, rr)
                        hook(f"sfc1_{chunks[0]}_{m}")

                def ga_late():
                    # second-half GA as soon as embT half 1 is live, so
                    # the expg broadcasts start mid phase A
                    ablock(GA_OFF, 10, ev_ga, (2, 3), None)
                    gate_head()

                if first_pass_blocks is not None:
                    # single-chunk prefix passes: start compute after only
                    # 4 gather tiles instead of 8
                    hooks.setdefault("sfc1_0_10", []).append(ga_late)
                    phaseA((0,), first_pass_blocks, ms=range(0, 6))
                    phaseA((1,), first_pass_blocks, ms=range(0, 6))
                    phaseA((0, 1), first_pass_blocks, ga=False, m1=False,
                           ms=range(6, 16))
                    hook("midA")
                    phaseA((2, 3), first_pass_blocks, ga=False)
                else:
                    hooks.setdefault("postGA0", []).append(gate_head)
                    phaseA((0, 1, 2, 3), ("A", "V"))
                hook("postA")

                # ---- gate normalizers (after expg complete) ----
                # corr = bsfc2 @ expg, only needed if any STT experts
                corr = None
                if STT_EXPERTS:
                    corr = wp.tile([128, BC], BF16, tag="corr", bufs=1,
                                   name=f"corr{s}")
                    for p in range(2):
                        ps = pA.tile([128, HB], F32, tag="A", name="ps_cor")
                        for h in range(2):
                            c0 = (2 * p + h) * CN
                            nc.tensor.matmul(ps[:, h * CN:(h + 1) * CN],
                                             w[0:11, BCOR_OFF:BCOR_OFF + 128],
                                             expg[0:11, c0:c0 + CN],
                                             start=True, stop=True)
                        nc.scalar.activation(corr[:, p * HB:(p + 1) * HB],
                                             ps[:], A.Copy)
                dbg("d_embT0", embT[:, :, :])
                dbg("d_expg0", expg[:, :])
                dbg("d_y1a0", y1a[:, :])
                dbg("d_s1r0", s1r[0][:, :])

                # gsel: [3, BC] rows S, A0, A1 in 4 chunk psums -> recips
                rows32 = wp.tile([3, BC], F32, tag="rows32", bufs=1,
                                 name=f"rows32_{s}")
                rows = wp.tile([3, BC], BF16, tag="rows", bufs=1,
                               name=f"rows{s}")
                for c in range(NCH):
                    ps = pC.tile([128, CN], F32, tag="C", name="ps_gs")
                    nc.tensor.matmul(ps[0:3, 0:CN],
                                     w[0:11, GSEL_OFF:GSEL_OFF + 3],
                                     expg[0:11, c * CN:(c + 1) * CN],
                                     start=True, stop=True)
                    nc.vector.reciprocal_approx_fast(
                        rows32[:, c * CN:(c + 1) * CN], ps[0:3, 0:CN])
                nc.scalar.activation(rows[:], rows32[:], A.Copy)
                dbg("d_rows0", rows32[:, :])
                nc.sync.dma_start(out=dram_bc[8:11, :], in_=rows[:, :])
                for r in range(3):
                    replicate(bR[:, r, :], 8 + r)
                for e in range(6, E):
                    replicate(bcg[e][:], e)
                dbg("d_bcg00", bcg[0][:, :])

                def kblock(wcols, movings, evict, mw=128):
                    # stationary-major: 2 psum tiles alternate; accumulate
                    # over k-chunks
                    pss = [pA.tile([128, HB], F32, tag="A", name="ps_k")
                           for _ in range(2)]
                    for kc, wc in enumerate(wcols):
                        for c in range(NCH):
                            ps = pss[c // 2]
                            nc.tensor.matmul(
                                ps[0:mw, (c % 2) * CN:(c % 2 + 1) * CN],
                                w[:, wc:wc + mw],
                                movings[kc][:, c * CN:(c + 1) * CN],
                                start=(kc == 0), stop=(kc == len(wcols) - 1))
                    fill(2)
                    for p in range(2):
                        evict(pss[p], p)

                # ---- m2 ----
                m2t = wp.tile([128, BC], BF16, tag="m2t", bufs=1,
                              name=f"m2t{s}")

                def ev_m2(ps, p):
                    if p == 0:
                        nc.scalar.activation(
                            m2t[:, 0:HB], ps[:], A.Relu,
                            bias=b[:, B2M_C:B2M_C + 1])
                    else:
                        nc.vector.tensor_scalar(
                            m2t[:, HB:BC], ps[:],
                            b[:, NB2M_C:NB2M_C + 1], b[:, B2M_C:B2M_C + 1],
                            op0=OP.max, op1=OP.add)

                kblock([W2_OFF, W2_OFF + 128], [y1a, y1b], ev_m2)
                dbg("d_m2t0", m2t[:, :])
                hook("postm2")

                # ---- sfc2 expert sweep + gated eager pairwise sum ----
                acc = wp.tile([128, BC], BF16, tag="acc", bufs=1,
                              name=f"acc{s}")
                pe_t = [wp.tile([128, BC], BF16, tag="pe", bufs=2,
                                name=f"pe{s}_{i}") for i in range(2)]
                for e in range(E):
                    pt = acc if e == 0 else pe_t[e % 2]

                    def ev_pe(ps, p, e=e, pt=pt):
                        # Pool cannot read PSUM. Experts 0-5: ACT relu
                        # eviction in place, then one full-width gate mul
                        # on Pool (e 0-2) / DVE (e 3-5). Experts 6-7:
                        # fused DVE scalar_tensor_tensor.
                        if e not in STT_EXPERTS:
                            nc.scalar.activation(
                                pt[:, p * HB:(p + 1) * HB], ps[:], A.Relu,
                                bias=b[:, PB2_C + e:PB2_C + e + 1])
                        else:
                            nc.vector.scalar_tensor_tensor(
                                pt[:, p * HB:(p + 1) * HB], ps[:],
                                b[:, NB2_C + e:NB2_C + e + 1],
                                bcg[e][:, p * HB:(p + 1) * HB],
                                op0=OP.max, op1=OP.mult)

                    kblock([SFC2_OFF + (2 * e) * 128,
                            SFC2_OFF + (2 * e + 1) * 128],
                           [s1r[2 * e], s1r[2 * e + 1]], ev_pe)
                    if e not in STT_EXPERTS:
                        # 2 muls on Pool: its ucode swap waits the gather
                        # drain (~late), and 3 serial 4us muls would set
                        # the add-chain endpoint
                        en = nc.gpsimd if e < 2 else nc.vector
                        en.tensor_mul(pt[:], pt[:], bcg[e][:])
                    if e > 0:
                        nc.vector.tensor_add(acc[:], acc[:], pt[:])
                    hook(f"sfc2_{e}")
                if corr is not None:
                    nc.vector.tensor_add(acc[:], acc[:], corr[:])
                dbg("d_acc0", acc[:, :])
                fill(16)  # keep PE warm through the share/sshare chain
                # spec emitted after the sweep: it waits on bR, and at the
                # head of the DVE queue it would block every sweep DVE op
                spec = wp.tile([128, BC], BF16, tag="spec", bufs=1,
                               name=f"spec{s}")
                nc.vector.tensor_mul(spec[:], m2t[:], bR[:, 2, :])
                dbg("d_spec0", spec[:, :])

                # share = acc * (1/S) * (1/A0) ; sshare = spec * share
                tmp = pe_t[0]
                nc.vector.tensor_mul(tmp[:], acc[:], bR[:, 0, :])
                share = wp.tile([128, BC], BF16, tag="share", bufs=1,
                                name=f"share{s}")
                nc.vector.tensor_mul(share[:], tmp[:], bR[:, 1, :])
                sshare = wp.tile([128, BC], BF16, tag="sshare", bufs=1,
                                 name=f"ssh{s}")
                nc.vector.tensor_mul(sshare[:], spec[:], share[:])
                dbg("d_share0", share[:, :])
                hook("postchain")
                return spec, share, sshare, nb

            def emit_tail(s, spec, share, sshare, nb, engs=("A", "A", "A")):
                # ---- m3 (1-bank psum chunks) + m4 + sout ----
                w, b, idx, ed, sout = sides[s]
                movs = ((spec, 0), (share, 2), (sshare, 1))
                h3a = wp.tile([128, BC], BF16, tag="h3a", bufs=1,
                              name=f"h3a{s}")
                h3b = wp.tile([64, BC], BF16, tag="h3b", bufs=1,
                              name=f"h3b{s}")

                def ev(eng, dst, ps, mw, bc):
                    if eng == "A":
                        nc.scalar.activation(dst, ps, A.Relu,
                                             bias=b[0:mw, bc:bc + 1])
                    else:
                        nc.vector.tensor_scalar(dst, ps,
                                                nb[0:mw, bc:bc + 1],
                                                b[0:mw, bc:bc + 1],
                                                op0=OP.max, op1=OP.add)

                for j, mw, dst in ((0, 128, h3a), (1, 64, h3b)):
                    for c in range(NCH):
                        ps = pC.tile([128, CN], F32, tag="C", name="ps_m3")
                        for ki, (mv, k) in enumerate(movs):
                            nc.tensor.matmul(
                                ps[0:mw, 0:CN],
                                w[:, W3_OFF + k * 192 + j * 128:
                                  W3_OFF + k * 192 + j * 128 + mw],
                                mv[:, c * CN:(c + 1) * CN],
                                start=(ki == 0), stop=(ki == 2))
                        ev(engs[j], dst[0:mw, c * CN:(c + 1) * CN],
                           ps[0:mw, 0:CN], mw, B3_C + j)

                ps4 = pC.tile([128, CN], F32, tag="C", name="ps_m4")
                for kc, (kw, wc, src) in enumerate(
                        ((128, W4_OFF, h3a), (64, W4_OFF + 32, h3b))):
                    for c in range(NCH):
                        nc.tensor.matmul(
                            ps4[32 * c:32 * c + 32, 0:CN],
                            w[0:kw, wc:wc + 32],
                            src[0:kw, c * CN:(c + 1) * CN],
                            start=(kc == 0), stop=(kc == 1),
                            tile_position=(0, 32 * c))
                ev(engs[2], sout[:], ps4[:, 0:CN], 128, B4_C)
                if DEBUG and s == 0:
                    nc.sync.dma_start(out=dbg_d["d_h3a0"][:, :], in_=h3a[:, :])
                if DEBUG:
                    nc.sync.dma_start(out=dbg_d[f"d_sout{s}"][:, :],
                                      in_=sout[:, :])

            # ================= global schedule =================
            # all gathers queued up front on gpsimd
            emit_gather(0, 0)
            emit_gather(0, 1)
            emit_gather(1, 0)
            emit_gather(1, 1)
            emit_xbar(0, 0, split=True)
            emit_xbar(0, 1)
            # gt pool bufs=2: side-1's gathers reuse side-0's slots only
            # after the side-0 xbars read them

            # side 0: 2-pass phase A (Pool busy with gathers -> ACT/DVE).
            # side-1 xbars emitted at postA: after gate_head's replicate
            # DMAs (sync queue) but before the bR replicates.
            r0 = emit_body(0, first_pass_blocks=("A", "V"), hooks={
                "midbcg": [lambda: [emit_xbar(1, h) for h in range(2)]]})
            # side 1: phase A first blocks fill PE while side0 chain runs
            tail0 = {"sfc1_0_2": (lambda: emit_tail(
                0, *r0, engs=("V", "A", "A")),)}
            r1 = emit_body(1, hooks=tail0)
            emit_tail(1, *r1, engs=("A", "A", "A"))

            # ---- logits: block-diagonal reduce of sout_u * sout_i ----
            su, si = sides[0][4], sides[1][4]
            prod = wp.tile([128, CN], BF16, tag="prod", bufs=1)
            nc.vector.tensor_mul(prod[:], su[:], si[:])
            psf = pC.tile([128, CN], F32, tag="C", name="ps_f")
            nc.tensor.matmul(psf[0:4, 0:CN],
                             sides[0][0][:, BLK_OFF:BLK_OFF + 4],
                             prod[:], start=True, stop=True)
            outsb = cp.tile([4, CN], F32)
            nc.scalar.activation(outsb[:], psf[0:4, 0:CN], A.Copy)
            nc.sync.dma_start(out=out_d[:, :], in_=outsb[:])

    nc.finalize()
    return nc


def _make_in_maps(inputs):
    d = int(inputs["domain_idc"])
    wu, bu = _fold_side(inputs, "u_", d, np.asarray(inputs["domain_embs"])[d])
    wi, bi = _fold_side(inputs, "i_", d, np.asarray(inputs["domain_embs"])[d + T])
    v = lambda x: x.view(np.uint16)
    embu = v(np.ascontiguousarray(np.asarray(inputs["emb_user"], np.float32).astype(BF)))
    embi = v(np.ascontiguousarray(np.asarray(inputs["emb_item"], np.float32).astype(BF)))
    uidx = np.asarray(inputs["user_indices"], np.int32)
    iidx = np.asarray(inputs["item_indices"], np.int32)

    in_maps = []
    for c in range(NCORES):
        sl = slice(c * BC, (c + 1) * BC)
        in_maps.append({
            "uidx": np.ascontiguousarray(uidx[sl].reshape(NT, 128).T),
            "iidx": np.ascontiguousarray(iidx[sl].reshape(NT, 128).T),
            "emb_user": embu,
            "emb_item": embi,
            "wu": v(wu), "wi": v(wi), "bu": bu, "bi": bi,
        })
    return in_maps


_CACHED_NC = None


def _get_nc():
    global _CACHED_NC
    if _CACHED_NC is None:
        _CACHED_NC = _build_nc()
    return _CACHED_NC


def run(inputs, **kw):
    """Run on 8 cores; returns (full_output, BassKernelResults)."""
    res = run_bass_kernel_spmd(_get_nc(), _make_in_maps(inputs),
                               core_ids=list(range(NCORES)), **kw)
    out = np.concatenate([res.results[c]["out"].reshape(-1)
                          for c in range(NCORES)])
    return out.reshape(B, 1).astype(np.float32), res


def kernel(**inputs):
    out, _ = run(inputs)
    return out
